# revision 1
# baseline (speedup 1.0000x reference)
"""Trainium2 Bass kernel for multi-query attention. (ORIGINAL BASELINE - 352575ns)

Problem: q [4,16,2048,64] f32, k/v [4,2048,64] f32 (KV shared across heads).
  out = softmax(q @ k^T / 8) @ v  ->  [4,16,2048,64] f32

Sharding (8 cores): batch x head-half. Core c handles batch c//2, heads
(c%2)*8 .. +8. k/v replicated per batch shard (they lack a head dim).

Device dataflow per core (ACT-bound design, engines at ~1.0-1.2 GHz):
  - Host pre-transposes q and k so d lands on SBUF partitions (free on host;
    avoids 144 PE transposes per core on device).
  - Per head-pair (A,B), i-block (512) and j-tile (128):
      2 row-packed fp32r matmuls (tile_position (0,0)/(64,0), K=64 each)
        -> S^T tile [128j, 2x512i] in 2 PSUM banks
      1 ACT exp over [128,1024] with scale=1/8 fused -> P^T f32r in SBUF
      2 fp32r matmuls accumulate O^T into per-i-block PSUM chains, with a
        ones-column appended to v so row 64 accumulates the softmax
        denominator (partition-dim reduction done by the PE for free).
  - Unnormalized [65, 2048] per head returned; host divides by the
    denominator row and transposes back. exp needs no max-subtraction:
    scores are ~N(0,1), |S|max ~ 6 over 268M samples, far from overflow.
"""

import numpy as np

B, H, N, D = 4, 16, 2048, 64
N_CORES = 8
HEADS_PER_CORE = H // 2   # 8
PAIRS = HEADS_PER_CORE // 2  # 4
JT = N // 128             # 16 j-tiles
IBLK = 4                  # i-blocks of 512
IW = 512


def _build_program(st_bufs=3, pt_bufs=6, o_bufs=1):
    import concourse.bacc as bacc
    import concourse.tile as tile
    import concourse.mybir as mybir

    f32 = mybir.dt.float32
    f32r = mybir.dt.float32r

    nc = bacc.Bacc("TRN2", target_bir_lowering=False, debug=False)
    qt_d = nc.dram_tensor("qt", [HEADS_PER_CORE, D, N], f32, kind="ExternalInput").ap()
    kt_d = nc.dram_tensor("kt", [D, N], f32, kind="ExternalInput").ap()
    v_d = nc.dram_tensor("v", [N, D], f32, kind="ExternalInput").ap()
    o_d = nc.dram_tensor("o", [HEADS_PER_CORE, D + 1, N], f32, kind="ExternalOutput").ap()

    with tile.TileContext(nc) as tc:
        with (
            tc.tile_pool(name="const", bufs=1) as cpool,
            tc.tile_pool(name="qstage", bufs=2) as qpool,
            tc.tile_pool(name="pt", bufs=pt_bufs) as ptpool,
            tc.tile_pool(name="osb", bufs=4) as opool,
            tc.tile_pool(name="spsum", bufs=st_bufs, space="PSUM") as spsum,
            tc.tile_pool(name="opsum", bufs=o_bufs, space="PSUM") as opsum,
        ):
            # Critical-path-first setup: pair 0's q and k load and round
            # before v, so the first QK->exp chunk starts ~10us earlier.
            qtmp0 = qpool.tile([128, N], f32, tag="qtmp", name="qtmp0")
            nc.sync.dma_start(qtmp0[0:D, 0:IW], qt_d[0][:, 0:IW])
            nc.sync.dma_start(qtmp0[D : 2 * D, 0:IW], qt_d[1][:, 0:IW])
            ktmp = cpool.tile([D, N], f32)
            nc.sync.dma_start(ktmp[:, 0:128], kt_d[:, 0:128])
            nc.sync.dma_start(ktmp[:, 128:], kt_d[:, 128:])
            # v's strided DMA (~3us) must beat the first AV matmul, so it
            # goes ahead of the non-critical remainder of q pair 0.
            vtmp = cpool.tile([128, JT, D], f32)
            nc.sync.dma_start(vtmp[:, 0:2, :], v_d[0:256].rearrange("(jt p) d -> p jt d", p=128))
            nc.sync.dma_start(qtmp0[0:D, IW:], qt_d[0][:, IW:])
            nc.sync.dma_start(qtmp0[D : 2 * D, IW:], qt_d[1][:, IW:])
            nc.sync.dma_start(vtmp[:, 2:JT, :], v_d[256:].rearrange("(jt p) d -> p jt d", p=128))

            # k^T, rounded to f32r, duplicated on partitions 64-127 so the
            # row-packed pair of matmuls can use row groups 0-1 and 2-3.
            # jt=0 slice rounds first to unblock the first matmul.
            ktr = cpool.tile([128, JT, 128], f32r)
            nc.vector.tensor_copy(ktr[0:D, 0, :], ktmp[:, 0:128])
            nc.vector.tensor_copy(ktr[D : 2 * D, 0, :], ktmp[:, 0:128])
            qtr0 = qpool.tile([128, IBLK, IW], f32r, tag="qtr", name="qtr0")
            nc.vector.tensor_copy(qtr0[:, 0, :], qtmp0[:, 0:IW])
            nc.vector.tensor_copy(ktr[0:D, 1:JT, :], ktmp[:, 128:].rearrange("p (jt j) -> p jt j", j=128))
            nc.vector.tensor_copy(ktr[D : 2 * D, 1:JT, :], ktmp[:, 128:].rearrange("p (jt j) -> p jt j", j=128))
            nc.vector.tensor_copy(
                qtr0[:, 1:IBLK, :].rearrange("p b i -> p (b i)"), qtmp0[:, IW:]
            )

            # v chunks in natural layout + ones column for the denominator.
            # jt 0-1 round first so the first AV matmuls aren't blocked on
            # the full-v rounding pass.
            vaug = cpool.tile([128, JT, D + 1], f32r)
            ones = cpool.tile([128, JT], f32)
            nc.gpsimd.memset(ones[:], 1.0)
            nc.vector.tensor_copy(vaug[:, 0:2, 0:D], vtmp[:, 0:2, :])
            nc.vector.tensor_copy(vaug[:, 0:2, D : D + 1], ones[:, 0:2].rearrange("p (jt o) -> p jt o", o=1))
            nc.vector.tensor_copy(vaug[:, 2:JT, 0:D], vtmp[:, 2:JT, :])
            nc.vector.tensor_copy(vaug[:, 2:JT, D : D + 1], ones[:, 2:JT].rearrange("p (jt o) -> p jt o", o=1))

            for pr in range(PAIRS):
                # Head pair staged with A on partitions 0-63, B on 64-127.
                if pr == 0:
                    qtr = qtr0
                else:
                    qtmp = qpool.tile([128, N], f32, tag="qtmp", name=f"qtmp{pr}")
                    nc.sync.dma_start(qtmp[0:D, :], qt_d[2 * pr])
                    nc.sync.dma_start(qtmp[D : 2 * D, :], qt_d[2 * pr + 1])
                    qtr = qpool.tile([128, IBLK, IW], f32r, tag="qtr", name=f"qtr{pr}")
                    nc.vector.tensor_copy(qtr[:].rearrange("p b i -> p (b i)"), qtmp[:])

                for ib in range(IBLK):
                    oa = opsum.tile([D + 1, IW], f32, tag="oa", name=f"oa{pr}_{ib}")
                    ob = opsum.tile([D + 1, IW], f32, tag="ob", name=f"ob{pr}_{ib}")
                    for jt in range(JT):
                        st = spsum.tile([128, 2, IW], f32, tag="st", name=f"st{pr}_{ib}_{jt}")
                        nc.tensor.matmul(
                            st[:, 0, :], ktr[0:D, jt, :], qtr[0:D, ib, :],
                            start=True, stop=True, tile_position=(0, 0),
                        )
                        nc.tensor.matmul(
                            st[:, 1, :], ktr[D : 2 * D, jt, :], qtr[D : 2 * D, ib, :],
                            start=True, stop=True, tile_position=(64, 0),
                        )
                        pt = ptpool.tile([128, 2, IW], f32r, tag="pt", name=f"pt{pr}_{ib}_{jt}")
                        nc.scalar.activation(
                            pt[:].rearrange("p h i -> p (h i)"),
                            st[:].rearrange("p h i -> p (h i)"),
                            mybir.ActivationFunctionType.Exp,
                            scale=float(D) ** -0.5,
                        )
                        nc.tensor.matmul(
                            oa[:], vaug[:, jt, :], pt[:, 0, :],
                            start=(jt == 0), stop=(jt == JT - 1),
                        )
                        nc.tensor.matmul(
                            ob[:], vaug[:, jt, :], pt[:, 1, :],
                            start=(jt == 0), stop=(jt == JT - 1),
                        )
                    osa = opool.tile([D + 1, IW], f32, tag="osa", name=f"osa{pr}_{ib}")
                    osb_t = opool.tile([D + 1, IW], f32, tag="osb", name=f"osb{pr}_{ib}")
                    nc.vector.tensor_copy(osa[:], oa[:])
                    nc.vector.tensor_copy(osb_t[:], ob[:])
                    nc.sync.dma_start(o_d[2 * pr, :, ib * IW : (ib + 1) * IW], osa[:])
                    nc.sync.dma_start(o_d[2 * pr + 1, :, ib * IW : (ib + 1) * IW], osb_t[:])
    nc.compile()
    return nc


_PROGRAM_CACHE = {}


def _get_program():
    if "nc" not in _PROGRAM_CACHE:
        _PROGRAM_CACHE["nc"] = _build_program()
    return _PROGRAM_CACHE["nc"]


def _make_in_maps(q, k, v):
    q = np.asarray(q, dtype=np.float32)
    k = np.asarray(k, dtype=np.float32)
    v = np.asarray(v, dtype=np.float32)
    qt = np.ascontiguousarray(q.transpose(0, 1, 3, 2))  # [B, H, D, N]
    kt = np.ascontiguousarray(k.transpose(0, 2, 1))     # [B, D, N]
    in_maps = []
    for c in range(N_CORES):
        b = c // 2
        h0 = (c % 2) * HEADS_PER_CORE
        in_maps.append(
            {
                "qt": np.ascontiguousarray(qt[b, h0 : h0 + HEADS_PER_CORE]),
                "kt": kt[b],
                "v": v[b],
            }
        )
    return in_maps


def _unpack(results):
    out = np.empty((B, H, N, D), dtype=np.float32)
    for c in range(N_CORES):
        b = c // 2
        h0 = (c % 2) * HEADS_PER_CORE
        o_un = results[c]["o"]  # [heads, D+1, N]
        o_n = o_un[:, :D, :] / o_un[:, D : D + 1, :]
        out[b, h0 : h0 + HEADS_PER_CORE] = o_n.transpose(0, 2, 1)
    return out


def kernel(q: np.ndarray, k: np.ndarray, v: np.ndarray) -> np.ndarray:
    from concourse.bass_utils import run_bass_kernel_spmd

    assert q.shape == (B, H, N, D) and k.shape == (B, N, D) and v.shape == (B, N, D)
    nc = _get_program()
    in_maps = _make_in_maps(q, k, v)
    res = run_bass_kernel_spmd(nc, in_maps, list(range(N_CORES)))
    return _unpack(res.results)

